# revision 1
# baseline (speedup 1.0000x reference)
"""Trainium2 Bass kernel for nn_AttentionMap (B=4, S=4096, D=256 full attention).

Sharding: 8 cores = 4 batches x 2 query-halves (data-parallel batch,
sequence-parallel over query rows, softmax rows stay whole per core).
No collectives: core c computes out[c//2, (c%2)*2048:(c%2+1)*2048, :]
from conv_local[c//2] and its conv_global slice.

Per-core algorithm (every matmul contracts over the partition dim):
  consts:  PE-transpose Wk, Wq; fuse the score weights once:
           M~T = (Wq^T-chunks) @ Wk^T  [g-feat, x-feat], b~ = Wk @ bq.
           bk is dropped entirely: it only adds a per-query-row constant
           to the scores, which softmax cancels exactly.
  phase 0+1 (fused pipeline over input chunks):
           load X chunk -> PE-transpose into XT [256,4096]
                        -> V chunk = X Wv + bv (+ ones cols, PSUM fp32)
           load G chunk -> PE-transpose into GT
                        -> YT tile = M~T.T @ GT + b~   [256,2048]
  phase 2: per q-tile of 512 query rows:
           S^T chunks [128s,512q] = XT_chunk^T @ YT_tile (PSUM fp32, pairs
             of chunks share one 2-bank PSUM tile)
           expS = exp(S^T / sqrt(256)) (ACT, one op per 2 chunks; no
             max-subtraction - scores ~ N(0,1) so fp32 exp is safe, and
             softmax is shift-invariant so results match the reference)
           O_unnorm[128q, 258] += expS_chunk^T @ V_chunk (4 PSUM
             accumulators, ones-columns of V give the softmax denominator)
           out = O_unnorm[:, :256] * reciprocal(O_unnorm[:, 256]) -> DMA.

ATTN_MM_MODE selects the PE operand dtype (PSUM accumulation is always
fp32): "bf16" (default, fastest: FWL + 1 cyc/row), "f32r" (fp32 storage
rounded to ~tf32 by producers; ~10x lower error, ~1.35x slower: 4-byte
weight loads do not overlap), "f16" (2-pass on this PE - slow), "f32"
(exact, 4 cyc/row). Measured end-to-end absmax relative error vs the fp32
reference: bf16 3.5e-3, f32r 3.7e-4, f32 1.8e-5.
"""

import os
import sys
from contextlib import ExitStack

import numpy as np

for _p in ("/opt/trn_rl_repo", "/root/.axon_site/_ro/trn_rl_repo"):
    if _p not in sys.path and os.path.isdir(_p):
        sys.path.append(_p)

import concourse.bass as bass
import concourse.mybir as mybir
import concourse.tile as tile
from concourse import bacc
from concourse.bass_utils import run_bass_kernel_spmd
from concourse.masks import make_identity

B = 4
S = 4096          # kv sequence length (= full query length)
D = 256           # model dim = head dim
NCORES = 8
SQH = S // 2      # query rows per core (2048)
QT = 512          # query tile (moving free dim of the S^T matmuls)
NQT = SQH // QT   # 4
NSC = S // 128    # 32 kv chunks of 128
NDC = D // 128    # 2 d chunks of 128
VPAD = 2          # ones-columns appended to V (even free dim for f32r matmul)
F32 = mybir.dt.float32
F32R = mybir.dt.float32r
BF16 = mybir.dt.bfloat16
F16 = mybir.dt.float16

# "bf16" (fast, default), "f32r" (precise+fast-ish), "f16" (slow), "f32" (exact)
MM_MODE = os.environ.get("ATTN_MM_MODE", "bf16")
ES_SPLIT = bool(int(os.environ.get("ATTN_ES_SPLIT", "0")))
PVLAG = int(os.environ.get("ATTN_PVLAG", "16"))  # S^T->PV pipeline lag, in pairs
# (16 = no interleave: PV after all S^T of the q-tile; fastest on HW for bf16)
BENCH_ALL = bool(int(os.environ.get("ATTN_BENCH_ALL", "0")))  # loop phases 0-2
NO_EXP = bool(int(os.environ.get("ATTN_NO_EXP", "0")))  # bench-only: DVE copy i/o exp
PV_QS = bool(int(os.environ.get("ATTN_PV_QS", "0")))  # PV loop: qs-outer (vs t-outer)
PV_CONST = bool(int(os.environ.get("ATTN_PV_CONST", "0")))  # bench-only: fixed PV lhsT
PV_OT = bool(int(os.environ.get("ATTN_PV_OT", "0")))  # PV computes O^T (vt stationary)
PV_DVE = bool(int(os.environ.get("ATTN_PV_DVE", "0")))  # denominators on DVE (needs PV_OT)
PV_P2 = bool(int(os.environ.get("ATTN_PV_P2", "0")))  # PV as 2-chains + DVE merge
ST1 = bool(int(os.environ.get("ATTN_ST1", "0")))  # single-bank S^T psum, per-chunk exp
DMA_TR = bool(int(os.environ.get("ATTN_DMA_TR", "0")))  # X/G transposes via xbar DMA (2-byte modes)
XPRE = int(os.environ.get("ATTN_XPRE", "4"))  # X tiles preloaded before const DMAs

_CACHED = {}


def build_program(bench_reps: int = 0):
    """bench_reps > 0 wraps phase 2 in a hardware For_i loop (timing only)."""
    nc = bacc.Bacc("TRN2", target_bir_lowering=False, debug=False)

    x_d = nc.dram_tensor("x", [S, D], F32, kind="ExternalInput").ap()
    g_d = nc.dram_tensor("g", [SQH, D], F32, kind="ExternalInput").ap()
    wk_d = nc.dram_tensor("wk", [D, D], F32, kind="ExternalInput").ap()
    wq_d = nc.dram_tensor("wq", [D, D], F32, kind="ExternalInput").ap()
    wv_d = nc.dram_tensor("wv", [D, D], F32, kind="ExternalInput").ap()
    bq_d = nc.dram_tensor("bq", [D, 1], F32, kind="ExternalInput").ap()
    bv_d = nc.dram_tensor("bv", [1, D], F32, kind="ExternalInput").ap()
    out_d = nc.dram_tensor("out", [SQH, D], F32, kind="ExternalOutput").ap()

    lowp = MM_MODE in ("bf16", "f16", "f32r")
    cast2b = MM_MODE in ("bf16", "f16")  # 2-byte modes: cast inputs pre-transpose
    sb_dt = {"f32": F32, "f32r": F32R, "bf16": BF16, "f16": F16}[MM_MODE]
    # dtype of the transpose datapath (input tiles + psum out must match)
    tr_dt = sb_dt if cast2b else F32

    with tile.TileContext(nc) as tc, ExitStack() as ctx:
        Copy = mybir.ActivationFunctionType.Copy
        Ident = mybir.ActivationFunctionType.Identity
        Exp = mybir.ActivationFunctionType.Exp

        consts = ctx.enter_context(tc.tile_pool(name="consts", bufs=1))
        big = ctx.enter_context(tc.tile_pool(name="big", bufs=1))

        ident = consts.tile([128, 128], tr_dt)
        make_identity(nc, ident[:])

        wk_sb = consts.tile([128, NDC, D], sb_dt)
        wq_sb = consts.tile([128, NDC, D], sb_dt)
        wv_sb = consts.tile([128, NDC, D], sb_dt)
        bq_sb = consts.tile([128, NDC, 1], F32)
        ones1 = consts.tile([1, 128], sb_dt)
        ones1_f32 = consts.tile([1, 128], F32)
        vone_f32 = consts.tile([128, NSC, VPAD], F32)
        bv_bc = consts.tile([128, D], F32)

        if XPRE:
            pre_ld = ctx.enter_context(tc.tile_pool(name="pre_ld", bufs=1))
            xld_pre = pre_ld.tile([128, XPRE, D], F32)
            for tp_ in range(XPRE):
                nc.sync.dma_start(xld_pre[:, tp_, :], x_d[tp_ * 128:(tp_ + 1) * 128, :])

        if lowp:
            wld = consts.tile([128, 3 * NDC, D], F32, tag="wld")
            for kc in range(NDC):
                nc.sync.dma_start(wld[:, 0 * NDC + kc, :], wk_d[kc * 128:(kc + 1) * 128, :])
                nc.sync.dma_start(wld[:, 1 * NDC + kc, :], wq_d[kc * 128:(kc + 1) * 128, :])
                nc.sync.dma_start(wld[:, 2 * NDC + kc, :], wv_d[kc * 128:(kc + 1) * 128, :])
            for kc in range(NDC):
                if cast2b:
                    nc.vector.tensor_copy(wk_sb[:, kc, :], wld[:, 0 * NDC + kc, :])
                nc.vector.tensor_copy(wq_sb[:, kc, :], wld[:, 1 * NDC + kc, :])
                nc.vector.tensor_copy(wv_sb[:, kc, :], wld[:, 2 * NDC + kc, :])
            bv_ld = consts.tile([1, D], F32, tag="bvl")
            nc.sync.dma_start(bv_ld[:], bv_d[:])
            bv_rhs = consts.tile([1, D], sb_dt, tag="bvc")
            nc.vector.tensor_copy(bv_rhs[:], bv_ld[:])
        else:
            for kc in range(NDC):
                nc.sync.dma_start(wk_sb[:, kc, :], wk_d[kc * 128:(kc + 1) * 128, :])
                nc.sync.dma_start(wq_sb[:, kc, :], wq_d[kc * 128:(kc + 1) * 128, :])
                nc.sync.dma_start(wv_sb[:, kc, :], wv_d[kc * 128:(kc + 1) * 128, :])
            bv_rhs = consts.tile([1, D], F32, tag="bvc")
            nc.sync.dma_start(bv_rhs[:], bv_d[:])
        for kc in range(NDC):
            nc.sync.dma_start(bq_sb[:, kc, :], bq_d[kc * 128:(kc + 1) * 128, :])
        ident_f32 = consts.tile([128, 128], F32, tag="idf32")
        if PV_OT and tr_dt != F32:
            make_identity(nc, ident_f32[:])
        one11 = consts.tile([1, 1], F32, tag="one11")
        nc.vector.memset(one11[:], 1.0)
        onecol_f32 = consts.tile([128, 1], F32, tag="onecol")
        nc.vector.memset(onecol_f32[:], 1.0)
        # memset on a float32r tile is invalid ISA; stage through f32 + copy
        nc.vector.memset(ones1_f32[:], 1.0)
        nc.vector.tensor_copy(ones1[:], ones1_f32[:])
        nc.vector.memset(vone_f32[:], 1.0)

        # ---- phase 2 SBUF residents (allocated first so they survive) ----
        # scores^T = XT.T @ YT where YT = Wk^T @ QhatT: the bk bias only adds
        # a per-query-row constant to scores, which softmax cancels exactly,
        # so K never needs to be materialized at all.
        xt = big.tile([128, NDC, S], sb_dt)       # X^T [d, s]
        yt = big.tile([128, NDC, SQH], sb_dt)     # Wk^T Qhat^T [d, q]
        vt = big.tile([128, NSC, D + VPAD], sb_dt)  # V||1 [s, d+pad]

        import contextlib
        bench_all = bool(bench_reps) and BENCH_ALL
        outer_cm = tc.For_i(0, bench_reps, 1) if bench_all else contextlib.nullcontext()
        p01_cm = ExitStack()
        outer_stack = ExitStack()
        outer_stack.enter_context(outer_cm)
        with p01_cm as p01:
            ld = p01.enter_context(tc.tile_pool(name="ld", bufs=8))
            trp = p01.enter_context(tc.tile_pool(name="trp", bufs=3, space="PSUM"))
            xtgt = p01.enter_context(tc.tile_pool(name="xtgt", bufs=1))
            mmp = p01.enter_context(tc.tile_pool(name="mmp", bufs=3, space="PSUM"))

            # bv broadcast across partitions via a K=1 matmul
            psb = mmp.tile([128, D], F32, tag="proj")
            nc.tensor.matmul(psb[:], ones1[:], bv_rhs[:], start=True, stop=True)
            nc.vector.tensor_copy(bv_bc[:], psb[:])

            gt = xtgt.tile([128, NDC, SQH], sb_dt)   # G^T [d, q]

            # Fused score weights: YT = M~T.T @ GT + b~ where
            # M~[a,i] = sum_dk Wk[a,dk] Wq[i,dk] (so M~T = Wq Wk^T viewed
            # [i,a]) and b~ = Wk @ bq.  This absorbs the whole Q projection.
            wkT_sb = consts.tile([128, NDC, D], sb_dt, tag="wkT")
            wqT_sb = consts.tile([128, NDC, D], sb_dt, tag="wqT")
            for a in range(NDC):
                for b in range(NDC):
                    pswt = trp.tile([128, 128], tr_dt, tag="tr", name="pswt")
                    if MM_MODE == "f32r":
                        wsrc = wld[:, 0 * NDC + b, a * 128:(a + 1) * 128]
                    else:
                        wsrc = wk_sb[:, b, a * 128:(a + 1) * 128]
                    nc.tensor.transpose(pswt[:], wsrc, ident[:])
                    nc.vector.tensor_copy(wkT_sb[:, a, b * 128:(b + 1) * 128], pswt[:])
                    psqt = trp.tile([128, 128], tr_dt, tag="tr", name="psqt")
                    if MM_MODE == "f32r":
                        qsrc = wld[:, 1 * NDC + b, a * 128:(a + 1) * 128]
                    else:
                        qsrc = wq_sb[:, b, a * 128:(a + 1) * 128]
                    nc.tensor.transpose(psqt[:], qsrc, ident[:])
                    nc.vector.tensor_copy(wqT_sb[:, a, b * 128:(b + 1) * 128], psqt[:])

            mt_sb = consts.tile([128, NDC, D], sb_dt, tag="mt")   # M~T [i, a]
            bt_sb = consts.tile([128, NDC, 1], F32, tag="bt")     # b~ [a]
            for ic in range(NDC):
                psm = mmp.tile([128, D], F32, tag="proj", name="psm")
                for dk in range(NDC):
                    nc.tensor.matmul(
                        psm[:],
                        wqT_sb[:, dk, ic * 128:(ic + 1) * 128],
                        wkT_sb[:, dk, :],
                        start=(dk == 0), stop=(dk == NDC - 1),
                    )
                nc.vector.tensor_copy(mt_sb[:, ic, :], psm[:])
            bq_c = consts.tile([128, NDC, 1], sb_dt, tag="bqc")
            for dk in range(NDC):
                nc.vector.tensor_copy(bq_c[:, dk, :], bq_sb[:, dk, :])
            for ac in range(NDC):
                psbt = mmp.tile([128, 1], F32, tag="proj", name="psbt")
                for dk in range(NDC):
                    nc.tensor.matmul(
                        psbt[:],
                        wkT_sb[:, dk, ac * 128:(ac + 1) * 128],
                        bq_c[:, dk, :],
                        start=(dk == 0), stop=(dk == NDC - 1),
                    )
                nc.vector.tensor_copy(bt_sb[:, ac, :], psbt[:])

            # ---- phases 0+1 fused: load + transpose + project per chunk ----
            # X chunks feed XT and the V-projection (per chunk)
            for t in range(NSC):
                if t < XPRE:
                    xld = xld_pre[:, t, :]
                else:
                    xld = ld.tile([128, D], F32, tag="ld")
                    nc.sync.dma_start(xld[:], x_d[t * 128:(t + 1) * 128, :])
                if cast2b:
                    xldc = ld.tile([128, D], sb_dt, tag="ldc")
                    nc.vector.tensor_copy(xldc[:], xld[:])
                    xsrc = xldc
                else:
                    xsrc = xld
                for kc in range(NDC):
                    if DMA_TR and cast2b:
                        nc.scalar.dma_start(xt[:, kc, t * 128:(t + 1) * 128],
                                            xsrc[:, kc * 128:(kc + 1) * 128],
                                            transpose=True)
                        continue
                    ps = trp.tile([128, 128], tr_dt, tag="tr")
                    nc.tensor.transpose(ps[:], xsrc[:, kc * 128:(kc + 1) * 128], ident[:])
                    if (t + kc) % 2 == 0:
                        nc.scalar.activation(xt[:, kc, t * 128:(t + 1) * 128], ps[:], Copy)
                    else:
                        nc.vector.tensor_copy(xt[:, kc, t * 128:(t + 1) * 128], ps[:])
                # V[t, :256] = X_t @ Wv + bv ; V[t, 256:] = 1
                psv = mmp.tile([128, D], F32, tag="proj", name="psv")
                for kc in range(NDC):
                    nc.tensor.matmul(
                        psv[:],
                        xt[:, kc, t * 128:(t + 1) * 128],
                        wv_sb[:, kc, :],
                        start=(kc == 0), stop=(kc == NDC - 1),
                    )
                nc.vector.tensor_add(vt[:, t, 0:D], psv[:], bv_bc[:])
            nc.vector.tensor_copy(vt[:, :, D:D + VPAD], vone_f32[:])

            # G chunks feed GT and YT (per group of 4 chunks)
            for t in range(SQH // 128):
                gld = ld.tile([128, D], F32, tag="ld")
                nc.sync.dma_start(gld[:], g_d[t * 128:(t + 1) * 128, :])
                if cast2b:
                    gldc = ld.tile([128, D], sb_dt, tag="ldc")
                    nc.vector.tensor_copy(gldc[:], gld[:])
                    gsrc = gldc
                else:
                    gsrc = gld
                for kc in range(NDC):
                    if DMA_TR and cast2b:
                        nc.scalar.dma_start(gt[:, kc, t * 128:(t + 1) * 128],
                                            gsrc[:, kc * 128:(kc + 1) * 128],
                                            transpose=True)
                        continue
                    ps = trp.tile([128, 128], tr_dt, tag="tr")
                    nc.tensor.transpose(ps[:], gsrc[:, kc * 128:(kc + 1) * 128], ident[:])
                    if (t + kc) % 2 == 0:
                        nc.scalar.activation(gt[:, kc, t * 128:(t + 1) * 128], ps[:], Copy)
                    else:
                        nc.vector.tensor_copy(gt[:, kc, t * 128:(t + 1) * 128], ps[:])
                if t % 4 == 3:
                    nt = t // 4
                    # YT[a, q] = sum_i M~T[i, a-block] @ GT[i, q] + b~[a]
                    for dc in range(NDC):
                        psy = mmp.tile([128, 512], F32, tag="proj", name="psy")
                        for ic in range(NDC):
                            nc.tensor.matmul(
                                psy[:],
                                mt_sb[:, ic, dc * 128:(dc + 1) * 128],
                                gt[:, ic, nt * 512:(nt + 1) * 512],
                                start=(ic == 0), stop=(ic == NDC - 1),
                            )
                        nc.vector.tensor_scalar_add(
                            yt[:, dc, nt * 512:(nt + 1) * 512], psy[:], bt_sb[:, dc, :])

        # ---- phase 2: attention ----
        es_bufs = 2 if cast2b else 1
        esp = ctx.enter_context(tc.tile_pool(name="esp", bufs=es_bufs))
        # each stp tile spans 2 PSUM banks so one ACTIVATE handles 2 kv-chunks
        # (ST1: single-bank tiles, 4 bufs, one ACTIVATE per chunk)
        stp = ctx.enter_context(tc.tile_pool(name="stp", bufs=(4 if ST1 else 2),
                                             space="PSUM"))
        pvp = ctx.enter_context(tc.tile_pool(name="pvp", bufs=1, space="PSUM"))
        osb_p = ctx.enter_context(tc.tile_pool(name="osb", bufs=4))

        inv_sqrt_d = 1.0 / float(np.sqrt(D))
        nqs = QT // 128
        HSC = NSC // 2
        if bench_reps and not bench_all:
            loop_cm = tc.For_i(0, bench_reps, 1)
        else:
            loop_cm = contextlib.nullcontext()
        with loop_cm:
            idf = ident_f32 if (PV_OT and tr_dt != F32) else ident
            emit_phase2(tc, nc, ctx, esp, stp, pvp, osb_p, xt, yt, vt, out_d,
                        sb_dt, inv_sqrt_d, nqs, HSC, Exp, idf, one11, onecol_f32)
        outer_stack.close()

    nc.compile()
    return nc


def emit_pv_chunk(nc, accs, halves, vt, t, nqs, HSC):
    eh = halves[t // HSC]
    for qs in range(nqs):
        lhsT = eh[:, 0, 0:128] if PV_CONST else eh[:, t % HSC, qs * 128:(qs + 1) * 128]
        nc.tensor.matmul(
            accs[qs][:],
            lhsT,
            vt[:, t, :],
            start=(t == 0), stop=(t == NSC - 1),
        )


def emit_phase2(tc, nc, ctx, esp, stp, pvp, osb_p, xt, yt, vt, out_d,
                sb_dt, inv_sqrt_d, nqs, HSC, Exp, idf=None, one11=None,
                onecol_f32=None):
    if True:
        for qi in range(NQT):
            q0 = qi * QT
            # es in two halves: frees the first half's WAR dependency midway
            # through the PV pass so the next q-tile's exp can start earlier
            if ES_SPLIT:
                es_a = esp.tile([128, HSC, QT], sb_dt, tag="esa", name="es_a")
                es_b = esp.tile([128, HSC, QT], sb_dt, tag="esb", name="es_b")
                halves = (es_a, es_b)
            else:
                es = esp.tile([128, NSC, QT], sb_dt, tag="es", name="es")
                halves = (es[:, 0:HSC, :], es[:, HSC:NSC, :])
            if PV_P2:
                acc_sb = osb_p.tile([128, nqs, D + VPAD], F32, tag="accsb",
                                    name="acc_sb", bufs=2)
                accs = None
            elif PV_OT:
                ot0 = pvp.tile([128, QT], F32, tag="ot0", name="ot0")
                ot1 = pvp.tile([128, QT], F32, tag="ot1", name="ot1")
                ots = (ot0, ot1)
                if not PV_DVE:
                    dn = pvp.tile([1, QT], F32, tag="dn", name="dn")
                accs = None
            else:
                accs = []
                for qs in range(nqs):
                    acc_t = pvp.tile([128, D + VPAD], F32, tag=f"acc{qs}", name=f"acc{qs}")
                    accs.append(acc_t)
            for tp in range(NSC // 2):
                if ST1:
                    for sub in range(2):
                        t = 2 * tp + sub
                        ps1 = stp.tile([128, QT], F32, tag="st", name="ps1")
                        for kc in range(NDC):
                            nc.tensor.matmul(
                                ps1[:],
                                xt[:, kc, t * 128:(t + 1) * 128],
                                yt[:, kc, q0:q0 + QT],
                                start=(kc == 0), stop=(kc == NDC - 1),
                            )
                        eh = halves[t // HSC]
                        nc.scalar.activation(eh[:, t % HSC, :], ps1[:], Exp,
                                             scale=inv_sqrt_d)
                else:
                    ps = stp.tile([128, 2 * QT], F32, tag="st")
                    for sub in range(2):
                        t = 2 * tp + sub
                        for kc in range(NDC):
                            nc.tensor.matmul(
                                ps[:, sub * QT:(sub + 1) * QT],
                                xt[:, kc, t * 128:(t + 1) * 128],
                                yt[:, kc, q0:q0 + QT],
                                start=(kc == 0), stop=(kc == NDC - 1),
                            )
                    eh = halves[(2 * tp) // HSC]
                    if NO_EXP:
                        nc.vector.tensor_copy(
                            eh[:, (2 * tp) % HSC:(2 * tp) % HSC + 2, :], ps[:])
                    else:
                        nc.scalar.activation(
                            eh[:, (2 * tp) % HSC:(2 * tp) % HSC + 2, :],
                            ps[:], Exp, scale=inv_sqrt_d)
                # software-pipelined PV: interleave with S^T so PE stays dense
                # while ACT works through the exp backlog (lag = PVLAG pairs)
                if (not PV_OT) and (not PV_P2) and tp >= PVLAG:
                    for t in (2 * (tp - PVLAG), 2 * (tp - PVLAG) + 1):
                        emit_pv_chunk(nc, accs, halves, vt, t, nqs, HSC)
            if PV_P2:
                # PV as independent 2-chains (the arrangement S^T proves is
                # fast): each psum pair covers 2 kv-chunks, DVE folds pairs
                # into an SBUF accumulator; V's ones-columns still carry the
                # softmax denominator.
                for tp in range(NSC // 2):
                    for qs in range(nqs):
                        pp = pvp.tile([128, D + VPAD], F32, tag="pvp2",
                                      name="pp", bufs=3)
                        for j in (0, 1):
                            t = 2 * tp + j
                            eh = halves[t // HSC]
                            nc.tensor.matmul(
                                pp[:],
                                eh[:, t % HSC, qs * 128:(qs + 1) * 128],
                                vt[:, t, :],
                                start=(j == 0), stop=(j == 1),
                            )
                        if tp == 0:
                            nc.vector.tensor_copy(acc_sb[:, qs, :], pp[:])
                        else:
                            nc.vector.tensor_add(acc_sb[:, qs, :], pp[:],
                                                 acc_sb[:, qs, :])
                for qs in range(nqs):
                    osb = osb_p.tile([128, D], F32, tag="osb")
                    rec = osb_p.tile([128, 1], F32, tag="rec")
                    nc.vector.reciprocal(rec[:], acc_sb[:, qs, D:D + 1])
                    nc.vector.tensor_scalar_mul(osb[:], acc_sb[:, qs, 0:D], rec[:])
                    nc.sync.dma_start(
                        out_d[q0 + qs * 128:q0 + (qs + 1) * 128, :], osb[:])
                continue
            if PV_OT:
                # O^T[dv, q] = sum_t V_t^T @ es_t: vt chunks are stationary
                # (128-col LDW amortized over a 512-wide stream), and the
                # ones-column of V (1-col LDW) yields the softmax denominator.
                if PV_DVE:
                    dsum = osb_p.tile([128, QT], F32, tag="dsum", name="dsum")
                for t in range(NSC):
                    esf = halves[t // HSC][:, t % HSC, :]
                    for dvc in range(2):
                        nc.tensor.matmul(
                            ots[dvc][:],
                            vt[:, t, dvc * 128:(dvc + 1) * 128],
                            esf,
                            start=(t == 0), stop=(t == NSC - 1),
                        )
                    if PV_DVE:
                        # partial denominators on the otherwise-idle DVE
                        if t == 0:
                            nc.vector.tensor_copy(dsum[:], esf)
                        else:
                            nc.vector.tensor_add(dsum[:], esf, dsum[:])
                    else:
                        nc.tensor.matmul(
                            dn[:],
                            vt[:, t, D:D + 1],
                            esf,
                            start=(t == 0), stop=(t == NSC - 1),
                        )
                otsb = osb_p.tile([128, 2, QT], F32, tag="otsb", name="otsb")
                nc.vector.tensor_copy(otsb[:, 0, :], ot0[:])
                nc.scalar.activation(otsb[:, 1, :], ot1[:],
                                     mybir.ActivationFunctionType.Copy)
                if PV_DVE:
                    # fold 128 partition-partials into the denominator row;
                    # reuse the just-released ot0 bank
                    dn = pvp.tile([1, QT], F32, tag="ot0", name="dn")
                    nc.tensor.matmul(dn[:], onecol_f32[:], dsum[:],
                                     start=True, stop=True)
                dnsb = osb_p.tile([1, QT], F32, tag="dnsb", name="dnsb")
                nc.vector.tensor_copy(dnsb[:], dn[:])
                recs = []
                for qs in range(nqs):
                    dnt = pvp.tile([128, 1], F32, tag="otr", name="dnt", bufs=2)
                    nc.tensor.matmul(dnt[:], dnsb[:, qs * 128:(qs + 1) * 128],
                                     one11[:], start=True, stop=True)
                    rec_t = osb_p.tile([128, 1], F32, tag=f"rec{qs}", name=f"rec{qs}")
                    nc.vector.reciprocal(rec_t[:], dnt[:])
                    recs.append(rec_t)
                for qs in range(nqs):
                    osb = osb_p.tile([128, D], F32, tag="osb")
                    for dvc in range(2):
                        otr = pvp.tile([128, 128], F32, tag="otr", name="otr",
                                       bufs=2)
                        nc.tensor.transpose(
                            otr[:], otsb[:, dvc, qs * 128:(qs + 1) * 128], idf[:])
                        nc.vector.tensor_scalar_mul(
                            osb[:, dvc * 128:(dvc + 1) * 128], otr[:], recs[qs][:])
                    nc.sync.dma_start(
                        out_d[q0 + qs * 128:q0 + (qs + 1) * 128, :], osb[:])
                continue
            if PV_QS and PVLAG >= NSC // 2:
                for qs in range(nqs):
                    for t in range(NSC):
                        eh = halves[t // HSC]
                        nc.tensor.matmul(
                            accs[qs][:],
                            eh[:, t % HSC, qs * 128:(qs + 1) * 128],
                            vt[:, t, :],
                            start=(t == 0), stop=(t == NSC - 1),
                        )
            else:
                for tp in range(NSC // 2 - PVLAG, NSC // 2):
                    for t in (2 * tp, 2 * tp + 1):
                        emit_pv_chunk(nc, accs, halves, vt, t, nqs, HSC)
            for qs in range(nqs):
                acc = accs[qs]
                osb = osb_p.tile([128, D], F32, tag="osb")
                rec = osb_p.tile([128, 1], F32, tag="rec")
                nc.vector.reciprocal(rec[:], acc[:, D:D + 1])
                nc.vector.tensor_scalar_mul(osb[:], acc[:, 0:D], rec[:])
                nc.sync.dma_start(
                    out_d[q0 + qs * 128:q0 + (qs + 1) * 128, :], osb[:]
                )


def _get_program():
    if "nc" not in _CACHED:
        _CACHED["nc"] = build_program()
    return _CACHED["nc"]


def kernel(conv_local, conv_global, Wk, bk, Wq, bq, Wv, bv):
    nc = _get_program()
    conv_local = np.ascontiguousarray(np.asarray(conv_local, dtype=np.float32))
    conv_global = np.ascontiguousarray(np.asarray(conv_global, dtype=np.float32))
    wk = np.ascontiguousarray(np.asarray(Wk, dtype=np.float32))
    wq = np.ascontiguousarray(np.asarray(Wq, dtype=np.float32))
    wv = np.ascontiguousarray(np.asarray(Wv, dtype=np.float32))
    bq = np.ascontiguousarray(np.asarray(bq, dtype=np.float32).reshape(D, 1))
    bv = np.ascontiguousarray(np.asarray(bv, dtype=np.float32).reshape(1, D))

    in_maps = []
    for c in range(NCORES):
        b, h = c // 2, c % 2
        in_maps.append({
            "x": conv_local[b],
            "g": np.ascontiguousarray(conv_global[b, h * SQH:(h + 1) * SQH]),
            "wk": wk, "wq": wq, "wv": wv,
            "bq": bq, "bv": bv,
        })

    trace = bool(int(os.environ.get("ATTN_TRACE", "0")))
    res = run_bass_kernel_spmd(nc, in_maps, list(range(NCORES)), trace=trace)
    _CACHED["last_results"] = res

    out = np.empty((B, S, D), dtype=np.float32)
    for c in range(NCORES):
        b, h = c // 2, c % 2
        out[b, h * SQH:(h + 1) * SQH] = res.results[c]["out"]
    return out



# revision 8
# speedup vs baseline: 11193.1927x; 11193.1927x over previous
"""Trainium2 Bass kernel for nn_AttentionMap (B=4, S=4096, D=256 full attention).

Sharding: 8 cores = 4 batches x 2 query-halves (data-parallel batch,
sequence-parallel over query rows, softmax rows stay whole per core).
No collectives: core c computes out[c//2, (c%2)*2048:(c%2+1)*2048, :]
from conv_local[c//2] and its conv_global slice.

Host-side preprocessing inside kernel() (same uploaded bytes, big device
savings):
  - X and G are uploaded TRANSPOSED (X^T [d, s], G^T [d, q]) so the device
    never runs the 96 PE transposes + 96 PSUM-drain copies the naive layout
    needs (they dominated the prologue).
  - The score weights are folded on the host in f32: M~ = Wq Wk^T [i, a]
    and b~ = Wk bq, so K and Q are never materialized; bk drops out
    entirely (softmax cancels a per-row constant).
  - All big tensors are cast to bf16 on the host (halves the axon-tunnel
    upload; the device would cast to bf16 for the PE anyway).

Device math (per core; every matmul contracts over the partition dim):
  Y^T[a, q] = M~T.T @ G^T + b~        (a-chunks of 128, q in tiles of 512)
  V[s, :256] = X^T-chunks.T @ Wv + bv ; V[s, 256:258] = 1  (ones-columns)
  per q-tile of 512 rows, software-pipelined with the previous tile's PV:
    S^T[s, q] = X^T-chunk.T @ Y^T-tile   (PSUM f32, 2 kv-chunks per 2-bank tile)
    p = exp(S^T / 16)  (ACT, bf16 out; no max-subtraction: scores ~ N(0,1)
        so bf16/f32 range is safe, softmax shift-invariance keeps results
        aligned with the reference)
    O_unnorm[q, 258] += p-chunk.T @ V-chunk  (4 PSUM accumulators; the
        ones-columns of V accumulate the softmax denominator)
    out = O_unnorm[:, :256] * reciprocal(O_unnorm[:, 256])  -> bf16 -> DMA
  The PV pass of q-tile i is interleaved between the S^T pairs of q-tile
  i+1 so the PE stays dense while ACT works through the exp backlog.

All matmuls are bf16 (fp8e4 DoubleRow was tried and is 1.9x faster on the
PE, but its ~3.6% operand quantization puts ~eps*w_max*|v| ~ 2-4e-2 absmax
error on rows where softmax concentrates - over the 2e-2 gate).

Output is bf16 (halves download), cast back to f32 on the host. The jitted
shard_map executable is built once and cached; donated output buffers are
created on-device instead of uploading host zeros.
"""

import os
import sys
from contextlib import ExitStack

import ml_dtypes
import numpy as np

for _p in ("/opt/trn_rl_repo", "/root/.axon_site/_ro/trn_rl_repo"):
    if _p not in sys.path and os.path.isdir(_p):
        sys.path.append(_p)

import concourse.mybir as mybir
import concourse.tile as tile
from concourse import bacc

B = 4
S = 4096          # kv sequence length (= full query length)
D = 256           # model dim = head dim
NCORES = 8
SQH = S // 2      # query rows per core (2048)
QT = 512          # query tile (moving free dim of the S^T matmuls)
NQT = SQH // QT   # 4
NSC = S // 128    # 32 kv chunks of 128
NDC = D // 128    # 2 d chunks of 128
VPAD = 2          # ones-columns appended to V (softmax denominator)
NXSEG = 8         # xt/gt DMA split for load/compute overlap
F32 = mybir.dt.float32
BF16 = mybir.dt.bfloat16

_CACHED = {}


def build_program():
    nc = bacc.Bacc("TRN2", target_bir_lowering=False, debug=False)

    xt_d = nc.dram_tensor("xt", [D, S], BF16, kind="ExternalInput").ap()
    gt_d = nc.dram_tensor("gt", [D, SQH], BF16, kind="ExternalInput").ap()
    mt_d = nc.dram_tensor("mt", [D, D], BF16, kind="ExternalInput").ap()
    wv_d = nc.dram_tensor("wv", [D, D], BF16, kind="ExternalInput").ap()
    bt_d = nc.dram_tensor("bt", [D, 1], F32, kind="ExternalInput").ap()
    bv_d = nc.dram_tensor("bv", [1, D], F32, kind="ExternalInput").ap()
    out_d = nc.dram_tensor("out", [SQH, D], BF16, kind="ExternalOutput").ap()

    with tile.TileContext(nc) as tc, ExitStack() as ctx:
        Exp = mybir.ActivationFunctionType.Exp

        consts = ctx.enter_context(tc.tile_pool(name="consts", bufs=1))
        big = ctx.enter_context(tc.tile_pool(name="big", bufs=1))

        # ---- phase 2 SBUF residents ----
        xt = big.tile([128, NDC, S], BF16)          # X^T [d, s]
        gt = big.tile([128, NDC, SQH], BF16)        # G^T [d, q]
        yt = big.tile([128, NDC, SQH], BF16)        # M~T.T G^T + b~  [a, q]
        vt = big.tile([128, NSC, D + VPAD], BF16)   # V||1 [s, d+pad]

        mt_sb = consts.tile([128, NDC, D], BF16)    # M~T [i, a]
        wv_sb = consts.tile([128, NDC, D], BF16)
        bt_sb = consts.tile([128, NDC, 1], F32)     # b~ [a]
        ones1 = consts.tile([1, 128], BF16)
        ones1_f32 = consts.tile([1, 128], F32)
        vone_f32 = consts.tile([128, NSC, VPAD], F32)
        bv_bc = consts.tile([128, D], F32)

        # xt/gt segment loads first so V/Y projections can start early
        xseg = S // NXSEG
        for kc in range(NDC):
            for sg in range(NXSEG):
                nc.sync.dma_start(
                    xt[:, kc, sg * xseg:(sg + 1) * xseg],
                    xt_d[kc * 128:(kc + 1) * 128, sg * xseg:(sg + 1) * xseg])
        gseg = SQH // NXSEG
        for kc in range(NDC):
            for sg in range(NXSEG):
                nc.sync.dma_start(
                    gt[:, kc, sg * gseg:(sg + 1) * gseg],
                    gt_d[kc * 128:(kc + 1) * 128, sg * gseg:(sg + 1) * gseg])
        for kc in range(NDC):
            nc.sync.dma_start(mt_sb[:, kc, :], mt_d[kc * 128:(kc + 1) * 128, :])
            nc.sync.dma_start(wv_sb[:, kc, :], wv_d[kc * 128:(kc + 1) * 128, :])
            nc.sync.dma_start(bt_sb[:, kc, :], bt_d[kc * 128:(kc + 1) * 128, :])
        bv_ld = consts.tile([1, D], F32, tag="bvl")
        nc.sync.dma_start(bv_ld[:], bv_d[:])
        bv_rhs = consts.tile([1, D], BF16, tag="bvc")
        nc.vector.tensor_copy(bv_rhs[:], bv_ld[:])
        nc.vector.memset(ones1_f32[:], 1.0)
        nc.vector.tensor_copy(ones1[:], ones1_f32[:])
        nc.vector.memset(vone_f32[:], 1.0)

        p01 = ExitStack()
        with p01:
            mmp = p01.enter_context(tc.tile_pool(name="mmp", bufs=4, space="PSUM"))

            # bv broadcast across partitions via a K=1 matmul
            psb = mmp.tile([128, D], F32, tag="proj")
            nc.tensor.matmul(psb[:], ones1[:], bv_rhs[:], start=True, stop=True)
            nc.vector.tensor_copy(bv_bc[:], psb[:])

            # V[t] = X_t @ Wv + bv ; ones-columns for the denominator
            for t in range(NSC):
                psv = mmp.tile([128, D], F32, tag="proj", name="psv")
                for kc in range(NDC):
                    nc.tensor.matmul(
                        psv[:],
                        xt[:, kc, t * 128:(t + 1) * 128],
                        wv_sb[:, kc, :],
                        start=(kc == 0), stop=(kc == NDC - 1),
                    )
                nc.vector.tensor_add(vt[:, t, 0:D], psv[:], bv_bc[:])
            nc.vector.tensor_copy(vt[:, :, D:D + VPAD], vone_f32[:])

            # Y^T[a, q] = sum_i M~T[i, a-block] @ G^T[i, q] + b~[a]
            for nt in range(SQH // 512):
                for dc in range(NDC):
                    psy = mmp.tile([128, 512], F32, tag="proj", name="psy")
                    for ic in range(NDC):
                        nc.tensor.matmul(
                            psy[:],
                            mt_sb[:, ic, dc * 128:(dc + 1) * 128],
                            gt[:, ic, nt * 512:(nt + 1) * 512],
                            start=(ic == 0), stop=(ic == NDC - 1),
                        )
                    nc.vector.tensor_scalar_add(
                        yt[:, dc, nt * 512:(nt + 1) * 512], psy[:], bt_sb[:, dc, :])

        # ---- phase 2: attention, PV software-pipelined one q-tile behind ----
        esp = ctx.enter_context(tc.tile_pool(name="esp", bufs=2))
        stp = ctx.enter_context(tc.tile_pool(name="stp", bufs=2, space="PSUM"))
        pvp = ctx.enter_context(tc.tile_pool(name="pvp", bufs=1, space="PSUM"))
        osb_p = ctx.enter_context(tc.tile_pool(name="osb", bufs=4))

        inv_sqrt_d = 1.0 / float(np.sqrt(D))
        nqs = QT // 128

        def emit_scores_pair(es, q0, tp):
            ps = stp.tile([128, 2, QT], F32, tag="st")
            for sub in range(2):
                t = 2 * tp + sub
                for kc in range(NDC):
                    nc.tensor.matmul(
                        ps[:, sub, :],
                        xt[:, kc, t * 128:(t + 1) * 128],
                        yt[:, kc, q0:q0 + QT],
                        start=(kc == 0), stop=(kc == NDC - 1),
                    )
            nc.scalar.activation(es[:, 2 * tp:2 * tp + 2, :], ps[:], Exp,
                                 scale=inv_sqrt_d)

        def emit_pv_pair(accs, es, tp):
            for qs in range(nqs):
                for j in (0, 1):
                    t = 2 * tp + j
                    nc.tensor.matmul(
                        accs[qs][:],
                        es[:, t, qs * 128:(qs + 1) * 128],
                        vt[:, t, :],
                        start=(t == 0), stop=(t == NSC - 1),
                    )

        def emit_finalize(accs, q0):
            for qs in range(nqs):
                acc = accs[qs]
                osb = osb_p.tile([128, D], BF16, tag="osb")
                rec = osb_p.tile([128, 1], F32, tag="rec")
                nc.vector.reciprocal(rec[:], acc[:, D:D + 1])
                nc.vector.tensor_scalar_mul(osb[:], acc[:, 0:D], rec[:])
                nc.sync.dma_start(
                    out_d[q0 + qs * 128:q0 + (qs + 1) * 128, :], osb[:])

        prev = None  # (es, accs, q0) of the q-tile whose PV is pending
        for qi in range(NQT):
            q0 = qi * QT
            es = esp.tile([128, NSC, QT], BF16, tag="es", name="es")
            accs = [pvp.tile([128, D + VPAD], F32, tag=f"acc{qs}", name=f"acc{qs}")
                    for qs in range(nqs)]
            for tp in range(NSC // 2):
                emit_scores_pair(es, q0, tp)
                if prev is not None:
                    emit_pv_pair(prev[1], prev[0], tp)
            if prev is not None:
                emit_finalize(prev[1], prev[2])
            prev = (es, accs, q0)
        for tp in range(NSC // 2):
            emit_pv_pair(prev[1], prev[0], tp)
        emit_finalize(prev[1], prev[2])

    nc.compile()
    return nc


def _get_exec():
    if "fn" in _CACHED:
        return _CACHED

    import jax
    import jax.numpy as jnp
    from jax.sharding import Mesh, PartitionSpec
    from jax.experimental.shard_map import shard_map
    from concourse import bass2jax

    nc = build_program()
    bass2jax.install_neuronx_cc_hook()

    partition_name = nc.partition_id_tensor.name if nc.partition_id_tensor else None
    in_names, out_names, out_avals = [], [], []
    for alloc in nc.m.functions[0].allocations:
        if not isinstance(alloc, mybir.MemoryLocationSet):
            continue
        name = alloc.memorylocations[0].name
        if alloc.kind == "ExternalInput":
            if name != partition_name:
                in_names.append(name)
        elif alloc.kind == "ExternalOutput":
            out_names.append(name)
            out_avals.append(jax.core.ShapedArray(
                tuple(alloc.tensor_shape), mybir.dt.np(alloc.dtype)))
    n_params = len(in_names)
    n_outs = len(out_avals)
    in_names_all = in_names + out_names
    if partition_name is not None:
        in_names_all.append(partition_name)
    donate = tuple(range(n_params, n_params + n_outs))

    def _body(*args):
        operands = list(args)
        if partition_name is not None:
            operands.append(bass2jax.partition_id_tensor())
        return tuple(bass2jax._bass_exec_p.bind(
            *operands,
            out_avals=tuple(out_avals),
            in_names=tuple(in_names_all),
            out_names=tuple(out_names),
            lowering_input_output_aliases=(),
            sim_require_finite=True,
            sim_require_nnan=True,
            nc=nc,
        ))

    devices = jax.devices()[:NCORES]
    mesh = Mesh(np.asarray(devices), ("core",))
    in_specs = (PartitionSpec("core"),) * (n_params + n_outs)
    out_specs = (PartitionSpec("core"),) * n_outs
    fn = jax.jit(
        shard_map(_body, mesh=mesh, in_specs=in_specs, out_specs=out_specs,
                  check_rep=False),
        donate_argnums=donate, keep_unused=True,
    )

    zero_info = [(tuple(a.shape), a.dtype) for a in out_avals]

    def _mkz():
        return tuple(jnp.zeros(shape, dtype) for shape, dtype in zero_info)

    zeros_fn = jax.jit(shard_map(
        _mkz, mesh=mesh, in_specs=(), out_specs=(PartitionSpec("core"),) * n_outs,
        check_rep=False))

    _CACHED.update(nc=nc, fn=fn, zeros_fn=zeros_fn, in_names=in_names,
                   out_names=out_names)
    return _CACHED


def kernel(conv_local, conv_global, Wk, bk, Wq, bq, Wv, bv):
    C = _get_exec()
    bf = ml_dtypes.bfloat16

    xl = np.asarray(conv_local, dtype=np.float32)
    xg = np.asarray(conv_global, dtype=np.float32)
    wk = np.asarray(Wk, dtype=np.float32)
    wq = np.asarray(Wq, dtype=np.float32)
    wv = np.asarray(Wv, dtype=np.float32)
    bqv = np.asarray(bq, dtype=np.float32).reshape(D)
    bvv = np.asarray(bv, dtype=np.float32).reshape(1, D)

    # core c = 2*b + h: X^T for batch b (repeated per half), G^T for half h
    xt = np.ascontiguousarray(xl.astype(bf).transpose(0, 2, 1))      # [B, D, S]
    xt_cat = np.repeat(xt, 2, axis=0).reshape(NCORES * D, S)
    gt = np.ascontiguousarray(
        xg.astype(bf).reshape(NCORES, SQH, D).transpose(0, 2, 1))    # [8, D, SQH]
    gt_cat = gt.reshape(NCORES * D, SQH)

    # fused score weights in f32 on the host: M~[i,a] = (Wq Wk^T), b~ = Wk bq
    mt = (wq @ wk.T).astype(bf)                                      # [i, a]
    bt = (wk @ bqv).astype(np.float32).reshape(D, 1)
    mt_cat = np.tile(mt, (NCORES, 1))
    bt_cat = np.tile(bt, (NCORES, 1))
    wv_cat = np.tile(wv.astype(bf), (NCORES, 1))
    bv_cat = np.tile(bvv, (NCORES, 1))

    arrs = {"xt": xt_cat, "gt": gt_cat, "mt": mt_cat, "wv": wv_cat,
            "bt": bt_cat, "bv": bv_cat}
    inputs = [arrs[name] for name in C["in_names"]]
    zeros = C["zeros_fn"]()
    outs = C["fn"](*inputs, *zeros)
    out = np.asarray(outs[0]).astype(np.float32)
    return out.reshape(B, 2, SQH, D).reshape(B, S, D)


# revision 12
# speedup vs baseline: 14750.8802x; 1.3178x over previous
"""Trainium2 Bass kernel for nn_AttentionMap (B=4, S=4096, D=256 full attention).

Sharding: 8 cores = 4 batches x 2 query-halves (data-parallel batch,
sequence-parallel over query rows, softmax rows stay whole per core).
No collectives: core c computes out[c//2, (c%2)*2048:(c%2+1)*2048, :]
from conv_local[c//2] and its conv_global slice.

Host-side preprocessing inside kernel() (same uploaded bytes, big device
savings):
  - X and G are uploaded TRANSPOSED (X^T [d, s], G^T [d, q]) so the device
    never runs the 96 PE transposes + 96 PSUM-drain copies the naive layout
    needs (they dominated the prologue).
  - The score weights are folded on the host in f32: M~ = Wq Wk^T [i, a]
    and b~ = Wk bq, so K and Q are never materialized; bk drops out
    entirely (softmax cancels a per-row constant).
  - All big tensors are cast to bf16 on the host (halves the axon-tunnel
    upload; the device would cast to bf16 for the PE anyway).

Device math (per core; every matmul contracts over the partition dim):
  Y^T[a, q] = M~T.T @ G^T + b~        (a-chunks of 128, q in tiles of 512)
  V[s, :256] = X^T-chunks.T @ Wv + bv ; V[s, 256:258] = 1  (ones-columns)
  per q-tile of 512 rows, software-pipelined with the previous tile's PV:
    S^T[s, q] = X^T-chunk.T @ Y^T-tile   (PSUM f32, 2 kv-chunks per 2-bank tile)
    p = exp(S^T / 16)  (ACT, bf16 out; no max-subtraction: scores ~ N(0,1)
        so bf16/f32 range is safe, softmax shift-invariance keeps results
        aligned with the reference)
    O_unnorm[q, 258] += p-chunk.T @ V-chunk  (4 PSUM accumulators; the
        ones-columns of V accumulate the softmax denominator)
    out = O_unnorm[:, :256] * reciprocal(O_unnorm[:, 256])  -> bf16 -> DMA
  The PV pass of q-tile i is interleaved between the S^T pairs of q-tile
  i+1 so the PE stays dense while ACT works through the exp backlog.

All matmuls are bf16 (fp8e4 DoubleRow was tried and is 1.9x faster on the
PE, but its ~3.6% operand quantization puts ~eps*w_max*|v| ~ 2-4e-2 absmax
error on rows where softmax concentrates - over the 2e-2 gate).

Output is bf16 (halves download), cast back to f32 on the host. The jitted
shard_map executable is built once and cached; donated output buffers are
created on-device instead of uploading host zeros.
"""

import os
import sys
from contextlib import ExitStack

import ml_dtypes
import numpy as np

for _p in ("/opt/trn_rl_repo", "/root/.axon_site/_ro/trn_rl_repo"):
    if _p not in sys.path and os.path.isdir(_p):
        sys.path.append(_p)

import concourse.mybir as mybir
import concourse.tile as tile
from concourse import bacc

B = 4
S = 4096          # kv sequence length (= full query length)
D = 256           # model dim = head dim
NCORES = 8
SQH = S // 2      # query rows per core (2048)
QT = 512          # query tile (moving free dim of the S^T matmuls)
NQT = SQH // QT   # 4
NSC = S // 128    # 32 kv chunks of 128
NDC = D // 128    # 2 d chunks of 128
VPAD = 2          # ones-columns appended to V (softmax denominator)
NXSEG = 8         # xt/gt DMA split for load/compute overlap
F32 = mybir.dt.float32
BF16 = mybir.dt.bfloat16

_CACHED = {}


def build_program():
    nc = bacc.Bacc("TRN2", target_bir_lowering=False, debug=False)

    xt_d = nc.dram_tensor("xt", [D, S], BF16, kind="ExternalInput").ap()
    gt_d = nc.dram_tensor("gt", [D, SQH], BF16, kind="ExternalInput").ap()
    mt_d = nc.dram_tensor("mt", [D, D], BF16, kind="ExternalInput").ap()
    wv_d = nc.dram_tensor("wv", [D, D], BF16, kind="ExternalInput").ap()
    bt_d = nc.dram_tensor("bt", [D, 1], F32, kind="ExternalInput").ap()
    bv_d = nc.dram_tensor("bv", [1, D], F32, kind="ExternalInput").ap()
    out_d = nc.dram_tensor("out", [SQH, D], BF16, kind="ExternalOutput").ap()

    with tile.TileContext(nc) as tc, ExitStack() as ctx:
        Exp = mybir.ActivationFunctionType.Exp

        consts = ctx.enter_context(tc.tile_pool(name="consts", bufs=1))
        big = ctx.enter_context(tc.tile_pool(name="big", bufs=1))

        # ---- phase 2 SBUF residents ----
        xt = big.tile([128, NDC, S], BF16)          # X^T [d, s]
        gt = big.tile([128, NDC, SQH], BF16)        # G^T [d, q]
        yt = big.tile([128, NDC, SQH], BF16)        # M~T.T G^T + b~  [a, q]
        vt = big.tile([128, NSC, D + VPAD], BF16)   # V||1 [s, d+pad]

        mt_sb = consts.tile([128, NDC, D], BF16)    # M~T [i, a]
        wv_sb = consts.tile([128, NDC, D], BF16)
        bt_sb = consts.tile([128, NDC, 1], F32)     # b~ [a]
        ones1 = consts.tile([1, 128], BF16)
        ones1_f32 = consts.tile([1, 128], F32)
        vone_f32 = consts.tile([128, NSC, VPAD], F32)
        bv_bc = consts.tile([128, D], F32)

        # weights first (small, unblock Y/V projections), then xt/gt segments
        # split across 4 DMA queues (sync/scalar/vector/tensor) — one queue
        # serialized the whole 3.4 MiB load at ~85 GB/s and stalled the
        # prologue for ~37 us
        bv_ld = consts.tile([1, D], F32, tag="bvl")
        nc.gpsimd.dma_start(bv_ld[:], bv_d[:])
        for kc in range(NDC):
            nc.gpsimd.dma_start(mt_sb[:, kc, :], mt_d[kc * 128:(kc + 1) * 128, :])
            nc.gpsimd.dma_start(wv_sb[:, kc, :], wv_d[kc * 128:(kc + 1) * 128, :])
            nc.gpsimd.dma_start(bt_sb[:, kc, :], bt_d[kc * 128:(kc + 1) * 128, :])
        xq = (nc.sync, nc.scalar)
        xseg = S // NXSEG
        gseg = SQH // NXSEG
        for sg in range(NXSEG):
            for kc in range(NDC):
                xq[kc].dma_start(
                    xt[:, kc, sg * xseg:(sg + 1) * xseg],
                    xt_d[kc * 128:(kc + 1) * 128, sg * xseg:(sg + 1) * xseg])
                nc.gpsimd.dma_start(
                    gt[:, kc, sg * gseg:(sg + 1) * gseg],
                    gt_d[kc * 128:(kc + 1) * 128, sg * gseg:(sg + 1) * gseg])
        bv_rhs = consts.tile([1, D], BF16, tag="bvc")
        nc.vector.tensor_copy(bv_rhs[:], bv_ld[:])
        nc.vector.memset(ones1_f32[:], 1.0)
        nc.vector.tensor_copy(ones1[:], ones1_f32[:])
        nc.vector.memset(vone_f32[:], 1.0)

        p01 = ExitStack()
        with p01:
            mmp = p01.enter_context(tc.tile_pool(name="mmp", bufs=4, space="PSUM"))

            # bv broadcast across partitions via a K=1 matmul
            psb = mmp.tile([128, D], F32, tag="proj")
            nc.tensor.matmul(psb[:], ones1[:], bv_rhs[:], start=True, stop=True)
            nc.vector.tensor_copy(bv_bc[:], psb[:])

            # V[t] = X_t @ Wv + bv ; ones-columns for the denominator
            for t in range(NSC):
                psv = mmp.tile([128, D], F32, tag="proj", name="psv")
                for kc in range(NDC):
                    nc.tensor.matmul(
                        psv[:],
                        xt[:, kc, t * 128:(t + 1) * 128],
                        wv_sb[:, kc, :],
                        start=(kc == 0), stop=(kc == NDC - 1),
                    )
                nc.vector.tensor_add(vt[:, t, 0:D], psv[:], bv_bc[:])
            nc.vector.tensor_copy(vt[:, :, D:D + VPAD], vone_f32[:])

            # Y^T[a, q] = sum_i M~T[i, a-block] @ G^T[i, q] + b~[a]
            for nt in range(SQH // 512):
                for dc in range(NDC):
                    psy = mmp.tile([128, 512], F32, tag="proj", name="psy")
                    for ic in range(NDC):
                        nc.tensor.matmul(
                            psy[:],
                            mt_sb[:, ic, dc * 128:(dc + 1) * 128],
                            gt[:, ic, nt * 512:(nt + 1) * 512],
                            start=(ic == 0), stop=(ic == NDC - 1),
                        )
                    nc.vector.tensor_scalar_add(
                        yt[:, dc, nt * 512:(nt + 1) * 512], psy[:], bt_sb[:, dc, :])

        # ---- phase 2: attention, PV software-pipelined one q-tile behind ----
        esp = ctx.enter_context(tc.tile_pool(name="esp", bufs=2))
        stp = ctx.enter_context(tc.tile_pool(name="stp", bufs=4, space="PSUM"))
        pvp = ctx.enter_context(tc.tile_pool(name="pvp", bufs=1, space="PSUM"))
        osb_p = ctx.enter_context(tc.tile_pool(name="osb", bufs=4))

        inv_sqrt_d = 1.0 / float(np.sqrt(D))
        nqs = QT // 128

        def emit_scores_pair(es, q0, tp):
            # single-bank psum tiles (bufs=4): 4-deep PE->ACT pipelining; a
            # 2-bank/bufs=2 pair tile ping-pongs PE and ACT (~12 us/qtile)
            for sub in range(2):
                t = 2 * tp + sub
                ps = stp.tile([128, QT], F32, tag="st")
                for kc in range(NDC):
                    nc.tensor.matmul(
                        ps[:],
                        xt[:, kc, t * 128:(t + 1) * 128],
                        yt[:, kc, q0:q0 + QT],
                        start=(kc == 0), stop=(kc == NDC - 1),
                    )
                nc.scalar.activation(es[:, t, :], ps[:], Exp, scale=inv_sqrt_d)

        def emit_pv_pair(accs, es, tp):
            for qs in range(nqs):
                for j in (0, 1):
                    t = 2 * tp + j
                    nc.tensor.matmul(
                        accs[qs][:],
                        es[:, t, qs * 128:(qs + 1) * 128],
                        vt[:, t, :],
                        start=(t == 0), stop=(t == NSC - 1),
                    )

        def emit_finalize(accs, q0):
            for qs in range(nqs):
                acc = accs[qs]
                osb = osb_p.tile([128, D], BF16, tag="osb")
                rec = osb_p.tile([128, 1], F32, tag="rec")
                nc.vector.reciprocal(rec[:], acc[:, D:D + 1])
                nc.vector.tensor_scalar_mul(osb[:], acc[:, 0:D], rec[:])
                nc.sync.dma_start(
                    out_d[q0 + qs * 128:q0 + (qs + 1) * 128, :], osb[:])

        prev = None  # (es, accs, q0) of the q-tile whose PV is pending
        for qi in range(NQT):
            q0 = qi * QT
            es = esp.tile([128, NSC, QT], BF16, tag="es", name="es")
            accs = [pvp.tile([128, D + VPAD], F32, tag=f"acc{qs}", name=f"acc{qs}")
                    for qs in range(nqs)]
            for tp in range(NSC // 2):
                emit_scores_pair(es, q0, tp)
                if prev is not None:
                    emit_pv_pair(prev[1], prev[0], tp)
            if prev is not None:
                emit_finalize(prev[1], prev[2])
            prev = (es, accs, q0)
        for tp in range(NSC // 2):
            emit_pv_pair(prev[1], prev[0], tp)
        emit_finalize(prev[1], prev[2])

    nc.compile()
    return nc


def _get_exec():
    if "fn" in _CACHED:
        return _CACHED

    import jax
    import jax.numpy as jnp
    from jax.sharding import Mesh, PartitionSpec
    from jax.experimental.shard_map import shard_map
    from concourse import bass2jax

    nc = build_program()
    bass2jax.install_neuronx_cc_hook()

    partition_name = nc.partition_id_tensor.name if nc.partition_id_tensor else None
    in_names, out_names, out_avals = [], [], []
    for alloc in nc.m.functions[0].allocations:
        if not isinstance(alloc, mybir.MemoryLocationSet):
            continue
        name = alloc.memorylocations[0].name
        if alloc.kind == "ExternalInput":
            if name != partition_name:
                in_names.append(name)
        elif alloc.kind == "ExternalOutput":
            out_names.append(name)
            out_avals.append(jax.core.ShapedArray(
                tuple(alloc.tensor_shape), mybir.dt.np(alloc.dtype)))
    n_params = len(in_names)
    n_outs = len(out_avals)
    in_names_all = in_names + out_names
    if partition_name is not None:
        in_names_all.append(partition_name)
    donate = tuple(range(n_params, n_params + n_outs))

    def _body(*args):
        operands = list(args)
        if partition_name is not None:
            operands.append(bass2jax.partition_id_tensor())
        return tuple(bass2jax._bass_exec_p.bind(
            *operands,
            out_avals=tuple(out_avals),
            in_names=tuple(in_names_all),
            out_names=tuple(out_names),
            lowering_input_output_aliases=(),
            sim_require_finite=True,
            sim_require_nnan=True,
            nc=nc,
        ))

    devices = jax.devices()[:NCORES]
    mesh = Mesh(np.asarray(devices), ("core",))
    in_specs = (PartitionSpec("core"),) * (n_params + n_outs)
    out_specs = (PartitionSpec("core"),) * n_outs
    fn = jax.jit(
        shard_map(_body, mesh=mesh, in_specs=in_specs, out_specs=out_specs,
                  check_rep=False),
        donate_argnums=donate, keep_unused=True,
    )

    zero_info = [(tuple(a.shape), a.dtype) for a in out_avals]

    def _mkz():
        return tuple(jnp.zeros(shape, dtype) for shape, dtype in zero_info)

    zeros_fn = jax.jit(shard_map(
        _mkz, mesh=mesh, in_specs=(), out_specs=(PartitionSpec("core"),) * n_outs,
        check_rep=False))

    _CACHED.update(nc=nc, fn=fn, zeros_fn=zeros_fn, in_names=in_names,
                   out_names=out_names)
    return _CACHED


def kernel(conv_local, conv_global, Wk, bk, Wq, bq, Wv, bv):
    C = _get_exec()
    bf = ml_dtypes.bfloat16

    xl = np.asarray(conv_local, dtype=np.float32)
    xg = np.asarray(conv_global, dtype=np.float32)
    wk = np.asarray(Wk, dtype=np.float32)
    wq = np.asarray(Wq, dtype=np.float32)
    wv = np.asarray(Wv, dtype=np.float32)
    bqv = np.asarray(bq, dtype=np.float32).reshape(D)
    bvv = np.asarray(bv, dtype=np.float32).reshape(1, D)

    # core c = 2*b + h: X^T for batch b (repeated per half), G^T for half h
    xt = np.ascontiguousarray(xl.astype(bf).transpose(0, 2, 1))      # [B, D, S]
    xt_cat = np.repeat(xt, 2, axis=0).reshape(NCORES * D, S)
    gt = np.ascontiguousarray(
        xg.astype(bf).reshape(NCORES, SQH, D).transpose(0, 2, 1))    # [8, D, SQH]
    gt_cat = gt.reshape(NCORES * D, SQH)

    # fused score weights in f32 on the host: M~[i,a] = (Wq Wk^T), b~ = Wk bq
    mt = (wq @ wk.T).astype(bf)                                      # [i, a]
    bt = (wk @ bqv).astype(np.float32).reshape(D, 1)
    mt_cat = np.tile(mt, (NCORES, 1))
    bt_cat = np.tile(bt, (NCORES, 1))
    wv_cat = np.tile(wv.astype(bf), (NCORES, 1))
    bv_cat = np.tile(bvv, (NCORES, 1))

    arrs = {"xt": xt_cat, "gt": gt_cat, "mt": mt_cat, "wv": wv_cat,
            "bt": bt_cat, "bv": bv_cat}
    inputs = [arrs[name] for name in C["in_names"]]
    zeros = C["zeros_fn"]()
    outs = C["fn"](*inputs, *zeros)
    out = np.asarray(outs[0]).astype(np.float32)
    return out.reshape(B, 2, SQH, D).reshape(B, S, D)
